# revision 16
# baseline (speedup 1.0000x reference)
"""Trainium2 Bass kernel for nn_DomainDiscriminator.

Network: conv(512->256,k3,s3,p1) -> BN -> conv(256->128,k3,s3,p1) -> BN
         -> reshape -> 12-layer MLP (3200->...->1, no nonlinearities) -> sigmoid.
Input x: [64, 512, 40, 40] f32.  Output: [64, 1] f32.

Strategy (8 NeuronCores, pure data parallel, 8 batch per core):
 - stride==kernel==3 convs are non-overlapping patch matmuls. Conv1 patches
   are built host-side (space-to-depth, bf16); conv2 patches are read out of
   SBUF with strided access patterns.
 - conv1 streams x in 18 small (k, half) tiles; the matmul loop is k-major
   within each half (pair of psum tiles per mt) so each weight tile is loaded
   once per 2 matmuls and the first matmul starts as soon as the first 392KB
   tile lands.
 - Training-mode BN: conv biases are absorbed exactly by BN; per-channel batch
   stats are computed per-psum-tile during conv1 (on DVE/ACT, which idle),
   exchanged with tiny 2KB AllGathers, tree-summed on chip.
 - ACT tables (Square/Sqrt/Sigmoid) are preloaded off the critical path;
   the BN2 AllGather is triggered before the weff partial matvec.
 - The 12 linear layers compose on the host (fp64) into a single [3200]
   vector + scalar bias; the device finishes with two tiny matmuls + sigmoid.
"""

import os
import sys

sys.path.insert(0, "/opt/trn_rl_repo")

import numpy as np

import concourse.bass as bass
import concourse.mybir as mybir
import concourse.tile as tile
from concourse import bacc
from concourse.bass_utils import run_bass_kernel_spmd

F32 = mybir.dt.float32
BF16 = mybir.dt.bfloat16

NCORES = 8
BL = 8              # batch per core
B = 64              # full batch
EPS = 1e-5

P1 = 196            # 14*14 conv1 positions
P2 = 25             # 5*5 conv2 positions
PTW = 2 * P1        # 392 cols per conv1 psum tile (2 batch)
N1 = float(B * P1)  # BN1 stat count
N2 = float(B * P2)  # BN2 stat count

_CACHE = {}

KIJ9 = [(ki, kj) for ki in range(3) for kj in range(3)]
# conv2 im2col: per (ki,kj) a full zero-padded 5*5*8 block in h1sb
BLK = 200
H1W = 9 * BLK       # 1800 cols per h1 tile


# ----------------------------------------------------------------------------
# device program
# ----------------------------------------------------------------------------

def _build():
    nc = bacc.Bacc("TRN2", target_bir_lowering=False, debug=False,
                   enable_asserts=True, num_devices=NCORES)

    # xprep: [9 kij, 2 half, 128, 4cb * 2ptin * 392]  (cb-major, ptin, cols)
    xprep = nc.dram_tensor("xprep", [9, 2, 128, 4 * 2 * PTW], BF16,
                           kind="ExternalInput")
    # w1p: [128, 9 kij, 4 cb, 256 m]
    w1p = nc.dram_tensor("w1p", [128, 9, 4 * 256], BF16, kind="ExternalInput")
    w2p = nc.dram_tensor("w2p", [128, 18, 128], BF16, kind="ExternalInput")
    # conv2 boundary-class row sums: [(class4, cb2) blocks][c1w, c2]
    w2sp = nc.dram_tensor("w2sp", [128, 8 * 128], BF16, kind="ExternalInput")
    weffp = nc.dram_tensor("weffp", [128, 26], F32, kind="ExternalInput")
    bprep = nc.dram_tensor("bprep", [128, 7], F32, kind="ExternalInput")
    out = nc.dram_tensor("out", [BL, 1], F32, kind="ExternalOutput")

    # bprep columns: bn1_g (2), bn1_b (2), bn2_g, bn2_b, beff(row 0)
    BC_BN1G, BC_BN1B, BC_BN2G, BC_BN2B, BC_BEFF = 0, 2, 4, 5, 6

    ISQ1 = 1.0 / np.sqrt(N1)   # Square-accum scale so accum = sum(h^2)/N1
    ISQ2 = 1.0 / np.sqrt(N2)

    with tile.TileContext(nc) as tc:
        with tc.tile_pool(name="wp", bufs=1) as wp, \
             tc.tile_pool(name="xp", bufs=5) as xp, \
             tc.tile_pool(name="hp", bufs=1) as hp, \
             tc.tile_pool(name="sp", bufs=1) as sp, \
             tc.tile_pool(name="cps", bufs=8, space="PSUM") as cps, \
             tc.tile_pool(name="dram", bufs=1, space="DRAM") as dram:

            # ---------------- first loads ------------------------------
            # Issue order: x(k0,h0) then w1(k0) so the first matmul is gated
            # only on ~520KB of cold DMA.
            w1sb = wp.tile([128, 9 * 1024], BF16)
            xt = {}
            for half in range(2):
                for k in range(9):
                    xt[(k, half)] = xp.tile([128, 8 * PTW], BF16,
                                            name=f"xt{k}_{half}", tag="xt")
            nc.sync.dma_start(xt[(0, 0)][:], xprep.ap()[0, 0])
            w1r = w1p.ap().rearrange("p a b -> p (a b)")
            nc.sync.dma_start(w1sb[:, 0:1024], w1r[:, 0:1024])

            # ncfw warm-up: a tiny AllGather nobody consumes; hides the
            # TOPSP cold-start under conv1
            warm_in = dram.tile([1, 4], F32)
            warm_out = dram.tile([NCORES, 1, 4], F32, addr_space="Shared")
            dummy = sp.tile([1, 8], F32)
            nc.gpsimd.memset(dummy[:], 0.0)
            epst = sp.tile([128, 1], F32)
            nc.gpsimd.memset(epst[:], EPS)
            nc.scalar.dma_start(warm_in[:], dummy[:, 0:4])
            nc.gpsimd.collective_compute(
                "AllGather", mybir.AluOpType.bypass,
                replica_groups=[list(range(NCORES))],
                ins=[warm_in.opt()], outs=[warm_out.opt()])
            # ACT Square table preload while ACT is idle
            nc.scalar.activation(dummy[:, 4:5], dummy[:, 5:6],
                                 mybir.ActivationFunctionType.Square)

            # remaining streamed loads, interleaved with compute demand order
            for k in range(1, 9):
                nc.sync.dma_start(xt[(k, 0)][:], xprep.ap()[k, 0])
                nc.sync.dma_start(w1sb[:, k * 1024:(k + 1) * 1024],
                                  w1r[:, k * 1024:(k + 1) * 1024])
            for k in range(9):
                nc.sync.dma_start(xt[(k, 1)][:], xprep.ap()[k, 1])
            w2sb = wp.tile([128, 18 * 128], BF16)
            nc.sync.dma_start(w2sb[:], w2p.ap().rearrange("p a b -> p (a b)"))
            w2s_sb = wp.tile([128, 8 * 128], BF16)
            nc.sync.dma_start(w2s_sb[:], w2sp.ap())
            weff = wp.tile([128, 26], F32)
            nc.sync.dma_start(weff[:], weffp.ap())
            bsb = wp.tile([128, 7], F32)
            nc.sync.dma_start(bsb[:], bprep.ap())

            # ---------------- conv1 (k-major per half) ------------------
            scratch = sp.tile([128, PTW], F32)
            # h1 tiles hold RAW conv1 output in zero-padded 5*5*8 blocks
            # per (ki,kj); border slots stay zero (memset once)
            h1sb = [hp.tile([128, H1W], BF16, name=f"h1_{mt}")
                    for mt in range(2)]
            for mt in range(2):
                nc.gpsimd.memset(h1sb[mt][:], 0.0)
            # per-psum-tile stats: [128, 4pt * (2kind*2mt)]; kind0=sum, 1=sumsq
            stt = sp.tile([128, 16], F32)

            for half in range(2):
                ps = {}
                for ptin in range(2):
                    for mt in range(2):
                        ps[(ptin, mt)] = cps.tile([128, PTW], F32,
                                                  name=f"c1ps{half}{ptin}{mt}",
                                                  tag="c1ps")
                for k in range(9):
                    xk = xt[(k, half)][:].rearrange(
                        "p (c t w) -> p c t w", c=4, t=2)
                    for cb in range(4):
                        for mt in range(2):
                            lhsT = w1sb[:, (k * 4 + cb) * 256 + mt * 128:
                                        (k * 4 + cb) * 256 + (mt + 1) * 128]
                            for ptin in range(2):
                                nc.tensor.matmul(
                                    ps[(ptin, mt)][:], lhsT, xk[:, cb, ptin],
                                    start=(k == 0 and cb == 0),
                                    stop=(k == 8 and cb == 3))
                for ptin in range(2):
                    pt = half * 2 + ptin
                    for mt in range(2):
                        p = ps[(ptin, mt)]
                        # stats straight off PSUM (f32): sum on DVE,
                        # sum(h^2)/N1 via ACT Square-accumulate
                        nc.vector.reduce_sum(stt[:, pt * 4 + mt:pt * 4 + mt + 1],
                                             p[:], axis=mybir.AxisListType.X)
                        nc.scalar.activation(scratch[:], p[:],
                                             mybir.ActivationFunctionType.Square,
                                             scale=ISQ1,
                                             accum_out=stt[:, pt * 4 + 2 + mt:
                                                           pt * 4 + 3 + mt])
                        # im2col blocks for conv2, cast to bf16
                        pr = p[:].rearrange("p (n i j) -> p n i j", n=2, i=14,
                                            j=14)
                        for kidx, (ki, kj) in enumerate(KIJ9):
                            ilo, icnt = (1, 4) if ki == 0 else (0, 5)
                            jlo, jcnt = (1, 4) if kj == 0 else (0, 5)
                            srcv = pr[:, :, 3 * ilo + ki - 1:14:3,
                                      3 * jlo + kj - 1:14:3].transpose([0, 2, 3, 1])
                            off = kidx * BLK + (ilo * 5 + jlo) * 8
                            dstv = bass.AP(
                                h1sb[mt].tensor,
                                h1sb[mt].offset + off + 2 * pt,
                                [list(h1sb[mt].ap[0]), [40, icnt],
                                 [8, jcnt], [1, 2]])
                            nc.vector.tensor_copy(dstv, srcv)

            # ---------------- BN1 stats combine + AllGather -------------
            # combine 4 pt blocks: [128, 4pt, 4] -> [128, 4]
            nc.vector.tensor_tensor(stt[:, 0:8], stt[:, 0:8], stt[:, 8:16],
                                    op=mybir.AluOpType.add)
            nc.vector.tensor_tensor(stt[:, 0:4], stt[:, 0:4], stt[:, 4:8],
                                    op=mybir.AluOpType.add)
            st1 = stt[:, 0:4]    # [S_mt0, S_mt1, Q_mt0, Q_mt1]; Q pre-div N1
            bn1_in = dram.tile([128, 4], F32)
            bn1_out = dram.tile([NCORES, 128, 4], F32, addr_space="Shared")
            nc.scalar.dma_start(bn1_in[:], st1)
            nc.gpsimd.collective_compute(
                "AllGather", mybir.AluOpType.bypass,
                replica_groups=[list(range(NCORES))],
                ins=[bn1_in.opt()], outs=[bn1_out.opt()])
            # Sqrt table preload during the AllGather wait; the stt read
            # keeps it ordered after the stats Squares, scale=0 + eps bias
            # keeps the argument in sqrt's valid range
            nc.scalar.activation(dummy[:, 4:5], stt[0:1, 0:1],
                                 mybir.ActivationFunctionType.Sqrt,
                                 scale=0.0, bias=epst[0:1, 0:1])
            stg = sp.tile([128, NCORES * 4], F32)
            nc.scalar.dma_start(
                stg[:].rearrange("p (r t) -> p r t", r=NCORES),
                bass.AP(bn1_out.tensor, 0, [[4, 128], [128 * 4, NCORES], [1, 4]]))
            stgr = stg[:].rearrange("p (r t) -> p r t", r=NCORES)
            for halfn in (4, 2, 1):
                nc.vector.tensor_tensor(
                    stgr[:, 0:halfn], stgr[:, 0:halfn],
                    stgr[:, halfn:2 * halfn], op=mybir.AluOpType.add)

            # ---------------- BN coeffs helper --------------------------
            def bn_coeffs(pool, stats_sum, stats_sqn, count, g_ap, b_ap, name):
                """stats_sum raw sum; stats_sqn = sum(h^2)/count.
                Returns (scale, shift) [p,w]."""
                p, w = stats_sum.shape
                t = pool.tile([p, 4 * w], F32, name=f"bn_{name}")
                mean, var, sd, tn = (t[:, i * w:(i + 1) * w] for i in range(4))
                nc.vector.tensor_scalar(mean, stats_sum, 1.0 / count, None,
                                        op0=mybir.AluOpType.mult)
                nc.vector.tensor_tensor(var, mean, mean,
                                        op=mybir.AluOpType.mult)
                nc.vector.tensor_tensor(var, stats_sqn, var,
                                        op=mybir.AluOpType.subtract)
                nc.scalar.activation(sd, var,
                                     mybir.ActivationFunctionType.Sqrt,
                                     bias=epst[:, 0:1])
                co = pool.tile([p, 3 * w], F32, name=f"bnc_{name}")
                scale, shift, r = co[:, 0:w], co[:, w:2 * w], co[:, 2 * w:3 * w]
                nc.vector.reciprocal(r, sd)
                nc.vector.tensor_tensor(scale, g_ap, r, op=mybir.AluOpType.mult)
                nc.vector.tensor_tensor(tn, mean, scale,
                                        op=mybir.AluOpType.mult)
                nc.vector.tensor_tensor(shift, b_ap, tn,
                                        op=mybir.AluOpType.subtract)
                return scale, shift

            scale1, shift1 = bn_coeffs(
                sp, stg[:, 0:2], stg[:, 2:4], N1,
                bsb[:, BC_BN1G:BC_BN1G + 2], bsb[:, BC_BN1B:BC_BN1B + 2], "bn1")
            # BN1 is folded into conv2: the per-input-channel scale goes
            # into w2 (h1sb's zero padding must stay zero), the shift's
            # contribution is a per-boundary-class constant computed by
            # 8 tiny matmuls against the host-precomputed class row sums.
            for cb2 in range(2):
                nc.vector.tensor_scalar(
                    w2sb[:, cb2 * 1152:(cb2 + 1) * 1152],
                    w2sb[:, cb2 * 1152:(cb2 + 1) * 1152],
                    scale1[:, cb2:cb2 + 1], None, op0=mybir.AluOpType.mult)
            shift1b = sp.tile([128, 2], BF16)
            nc.vector.tensor_copy(shift1b[:], shift1)
            Tm = cps.tile([128, 4], F32, tag="c1ps")
            for cls in range(4):
                for cb2 in range(2):
                    nc.tensor.matmul(Tm[:, cls:cls + 1],
                                     w2s_sb[:, (cls * 2 + cb2) * 128:
                                            (cls * 2 + cb2 + 1) * 128],
                                     shift1b[:, cb2:cb2 + 1],
                                     start=(cb2 == 0), stop=(cb2 == 1),
                                     skip_group_check=True)
            TmS = sp.tile([128, 4], F32)
            nc.vector.tensor_copy(TmS[:], Tm[:])

            # ---------------- conv2 (full 200-col blocks) ---------------
            c2ps = []
            for cb2 in range(2):
                cp = cps.tile([128, P2 * BL], F32, name=f"c2ps{cb2}",
                              tag="c1ps")
                c2ps.append(cp)
                for kidx in range(9):
                    lhsT = w2sb[:, (cb2 * 9 + kidx) * 128:
                                (cb2 * 9 + kidx + 1) * 128]
                    nc.tensor.matmul(cp[:], lhsT,
                                     h1sb[cb2][:, kidx * BLK:(kidx + 1) * BLK],
                                     start=(kidx == 0), stop=(kidx == 8))
            # DVE has a single PSUM read port: go through SBUF for the add
            c2half = sp.tile([128, BL * P2], F32)
            nc.vector.tensor_copy(c2half[:], c2ps[0][:])
            c2sb = sp.tile([128, BL * P2], BF16)
            nc.vector.tensor_tensor(c2sb[:], c2half[:], c2ps[1][:],
                                    op=mybir.AluOpType.add)
            # add the BN1-shift boundary-class constants
            c2v = c2sb[:].rearrange("p (i j n) -> p i j n", i=5, j=5, n=BL)
            for cls, view in ((3, c2v[:, 0:1, 0:1]), (2, c2v[:, 0:1, 1:5]),
                              (1, c2v[:, 1:5, 0:1]), (0, c2v[:, 1:5, 1:5])):
                nc.vector.tensor_scalar(view, view, TmS[:, cls:cls + 1], None,
                                        op0=mybir.AluOpType.add)

            # ---------------- BN2 stats + AllGather (before matvec) -----
            st2l = sp.tile([128, 2], F32)
            nc.vector.reduce_sum(st2l[:, 0:1], c2sb[:], axis=mybir.AxisListType.X)
            sc2 = sp.tile([128, BL * P2], F32)
            nc.scalar.activation(sc2[:], c2sb[:],
                                 mybir.ActivationFunctionType.Square,
                                 scale=ISQ2, accum_out=st2l[:, 1:2])
            bn2_in = dram.tile([128, 2], F32)
            bn2_out = dram.tile([NCORES, 128, 2], F32, addr_space="Shared")
            nc.scalar.dma_start(bn2_in[:], st2l[:])
            nc.gpsimd.collective_compute(
                "AllGather", mybir.AluOpType.bypass,
                replica_groups=[list(range(NCORES))],
                ins=[bn2_in.opt()], outs=[bn2_out.opt()])

            # during the AllGather: weff partial matvec + Sigmoid table +
            # the pieces of the finish that don't need stats
            mvt = sp.tile([128, P2 * BL], F32)
            wb = weff[:, 0:25, None].to_broadcast([128, 25, BL])
            nc.vector.tensor_tensor(
                mvt[:].rearrange("p (i n) -> p i n", i=P2),
                c2sb[:].rearrange("p (i n) -> p i n", i=P2), wb,
                op=mybir.AluOpType.mult)
            Av = sp.tile([128, BL], F32)
            nc.vector.reduce_sum(Av[:], mvt[:].rearrange("p (i n) -> p n i", i=P2),
                                 axis=mybir.AxisListType.X)
            Avb = sp.tile([128, BL], BF16)
            nc.vector.tensor_copy(Avb[:], Av[:])
            nc.scalar.activation(dummy[:, 4:5], st2l[0:1, 0:1],
                                 mybir.ActivationFunctionType.Sigmoid)
            ones = wp.tile([128, BL], BF16)
            nc.gpsimd.memset(ones[:], 1.0)

            stg2 = sp.tile([128, NCORES * 2], F32)
            nc.scalar.dma_start(
                stg2[:].rearrange("p (r t) -> p r t", r=NCORES),
                bass.AP(bn2_out.tensor, 0, [[2, 128], [128 * 2, NCORES], [1, 2]]))
            stg2r = stg2[:].rearrange("p (r t) -> p r t", r=NCORES)
            for halfn in (4, 2, 1):
                nc.vector.tensor_tensor(
                    stg2r[:, 0:halfn], stg2r[:, 0:halfn],
                    stg2r[:, halfn:2 * halfn], op=mybir.AluOpType.add)
            scale2, shift2 = bn_coeffs(
                sp, stg2[:, 0:1], stg2[:, 1:2], N2,
                bsb[:, BC_BN2G:BC_BN2G + 1], bsb[:, BC_BN2B:BC_BN2B + 1], "bn2")

            # ---------------- collapsed MLP finish ----------------------
            # z[n] = sum_c s2[c]*A[c,n] + sum_c shift2[c]*rowsum_weff[c]
            s2b = sp.tile([128, 1], BF16)
            nc.vector.tensor_copy(s2b[:], scale2)
            vsh = wp.tile([128, 1], BF16)
            nc.vector.tensor_tensor(vsh[:], shift2, weff[:, 25:26],
                                    op=mybir.AluOpType.mult)
            zps = cps.tile([1, BL], F32, tag="c1ps")
            nc.tensor.matmul(zps[:], s2b[:], Avb[:], start=True, stop=False)
            nc.tensor.matmul(zps[:], vsh[:], ones[:], start=False, stop=True)
            osb = sp.tile([1, BL], F32)
            nc.scalar.activation(osb[:], zps[:],
                                 mybir.ActivationFunctionType.Sigmoid,
                                 bias=bsb[0:1, BC_BEFF:BC_BEFF + 1])
            nc.sync.dma_start(bass.AP(out, 0, [[1, 1], [1, BL]]), osb[:])

    nc.compile()
    return nc


# ----------------------------------------------------------------------------
# host-side input prep
# ----------------------------------------------------------------------------

def _prep_inputs(inputs):
    import ml_dtypes
    f = np.float32
    bf = ml_dtypes.bfloat16
    x = np.asarray(inputs["x"], dtype=f)

    # conv1 patches: [n64, cb4, c128, i14, ki3, j14, kj3]
    xpad = np.zeros((B, 512, 42, 42), dtype=bf)
    xpad[:, :, 1:41, 1:41] = x.astype(bf)
    # -> [k9, cb4, c128, n64, pos196]
    xv = (xpad.reshape(B, 4, 128, 14, 3, 14, 3)
          .transpose(4, 6, 1, 2, 0, 3, 5)        # ki,kj,cb,c,n,i,j
          .reshape(9, 4, 128, B, P1))

    w1 = np.asarray(inputs["conv1_w"], dtype=f)          # [256, 512, 3, 3]
    # [128c, 9k, 4cb, 256m]
    w1p = np.ascontiguousarray(
        w1.reshape(256, 4, 128, 9).transpose(2, 3, 1, 0)).reshape(
            128, 9, 1024).astype(bf)
    w2 = np.asarray(inputs["conv2_w"], dtype=f)          # [128, 256, 3, 3]
    w2p = np.ascontiguousarray(
        w2.reshape(128, 2, 128, 9).transpose(2, 1, 3, 0)).reshape(
            128, 18, 128).astype(bf)
    # boundary-class row sums for the BN1-shift term:
    # class c = a*2+b, a=(i==0) -> ki>=1 only, b=(j==0) -> kj>=1 only
    w2r = w2.reshape(128, 2, 128, 3, 3)                  # c2, cb, c1w, ki, kj
    w2sp = np.zeros((128, 8 * 128), dtype=f)
    for cls in range(4):
        a, b = cls // 2, cls % 2
        kis = slice(1, 3) if a else slice(0, 3)
        kjs = slice(1, 3) if b else slice(0, 3)
        s = w2r[:, :, :, kis, kjs].sum(axis=(3, 4))      # c2, cb, c1w
        for cb in range(2):
            w2sp[:, (cls * 2 + cb) * 128:(cls * 2 + cb + 1) * 128] = s[:, cb].T
    w2sp = w2sp.astype(bf)

    # compose the 12 affine layers (no nonlinearities) into [3200] + scalar
    M = np.asarray(inputs["w14"], dtype=np.float64)      # [1, 2]
    beff = np.asarray(inputs["b14"], dtype=np.float64).copy()  # [1]
    for li in range(13, 2, -1):                          # w13 .. w3
        beff += M @ np.asarray(inputs[f"b{li}"], dtype=np.float64)
        M = M @ np.asarray(inputs[f"w{li}"], dtype=np.float64)
    weff = M.reshape(3200).astype(f)                     # order f = c*25 + ij
    w2d = weff.reshape(128, 25)
    weffp = np.zeros((128, 26), dtype=f)
    weffp[:, 0:25] = w2d
    weffp[:, 25] = w2d.sum(axis=1)
    beff_f = float(beff[0])

    bp = np.zeros((128, 7), dtype=f)
    bp[:, 0:2] = np.asarray(inputs["bn1_g"], dtype=f).reshape(2, 128).T
    bp[:, 2:4] = np.asarray(inputs["bn1_b"], dtype=f).reshape(2, 128).T
    bp[:, 4] = np.asarray(inputs["bn2_g"], dtype=f)
    bp[:, 5] = np.asarray(inputs["bn2_b"], dtype=f)
    bp[0, 6] = beff_f

    in_maps = []
    for r in range(NCORES):
        # [9k, 4cb, 128, 8n, 196] -> [9k, 2half, 128, 4cb, 2ptin, 2n, 196]
        xr = np.ascontiguousarray(
            xv[:, :, :, r * BL:(r + 1) * BL]
            .reshape(9, 4, 128, 2, 2, 2, P1)     # k, cb, c, half, ptin, n2, pos
            .transpose(0, 3, 2, 1, 4, 5, 6)
        ).reshape(9, 2, 128, 4 * 2 * PTW)
        in_maps.append({
            "xprep": xr, "w1p": w1p, "w2p": w2p, "w2sp": w2sp,
            "weffp": weffp, "bprep": bp,
        })
    return in_maps


def kernel(**inputs):
    if "nc" not in _CACHE:
        _CACHE["nc"] = _build()
    nc = _CACHE["nc"]
    in_maps = _prep_inputs(inputs)
    trace = bool(int(os.environ.get("KERNEL_TRACE", "0")))
    if trace:
        try:
            import ntff_shim
            ntff_shim.install()
        except ImportError:
            trace = False
    res = run_bass_kernel_spmd(nc, in_maps, core_ids=list(range(NCORES)),
                               trace=trace)
    _CACHE["last_result"] = res
    return np.concatenate([res.results[r]["out"] for r in range(NCORES)], axis=0)


# revision 19
# speedup vs baseline: 1.0339x; 1.0339x over previous
"""Trainium2 Bass kernel for nn_DomainDiscriminator.

Network: conv(512->256,k3,s3,p1) -> BN -> conv(256->128,k3,s3,p1) -> BN
         -> reshape -> 12-layer MLP (3200->...->1, no nonlinearities) -> sigmoid.
Input x: [64, 512, 40, 40] f32.  Output: [64, 1] f32.

Strategy (8 NeuronCores, pure data parallel, 8 batch per core):
 - stride==kernel==3 convs are non-overlapping patch matmuls. Conv1 patches
   are built host-side (space-to-depth, bf16); conv2 patches are read out of
   SBUF with strided access patterns.
 - conv1 streams x in 18 small (k, half) tiles; the matmul loop is k-major
   within each half (pair of psum tiles per mt) so each weight tile is loaded
   once per 2 matmuls and the first matmul starts as soon as the first 392KB
   tile lands.
 - Training-mode BN: conv biases are absorbed exactly by BN; per-channel batch
   stats are computed per-psum-tile during conv1 (on DVE/ACT, which idle),
   exchanged with tiny 2KB AllGathers, tree-summed on chip.
 - ACT tables (Square/Sqrt/Sigmoid) are preloaded off the critical path;
   the BN2 AllGather is triggered before the weff partial matvec.
 - The 12 linear layers compose on the host (fp64) into a single [3200]
   vector + scalar bias; the device finishes with two tiny matmuls + sigmoid.
"""

import os
import sys

sys.path.insert(0, "/opt/trn_rl_repo")

import numpy as np

import concourse.bass as bass
import concourse.mybir as mybir
import concourse.tile as tile
from concourse import bacc
from concourse.bass_utils import run_bass_kernel_spmd

F32 = mybir.dt.float32
BF16 = mybir.dt.bfloat16

NCORES = 8
BL = 8              # batch per core
B = 64              # full batch
EPS = 1e-5

P1 = 196            # 14*14 conv1 positions
P2 = 25             # 5*5 conv2 positions
PTW = 2 * P1        # 392 cols per conv1 psum tile (2 batch)
N1 = float(B * P1)  # BN1 stat count
N2 = float(B * P2)  # BN2 stat count

_CACHE = {}

KIJ9 = [(ki, kj) for ki in range(3) for kj in range(3)]
# conv2 im2col: per (ki,kj) a full zero-padded 5*5*8 block in h1sb
BLK = 200
H1W = 9 * BLK       # 1800 cols per h1 tile


# ----------------------------------------------------------------------------
# device program
# ----------------------------------------------------------------------------

def _build():
    nc = bacc.Bacc("TRN2", target_bir_lowering=False, debug=False,
                   enable_asserts=True, num_devices=NCORES)

    # xprep: [9 kij, 2 half, 128, 4cb * 2ptin * 392]  (cb-major, ptin, cols)
    xprep = nc.dram_tensor("xprep", [9, 2, 128, 4 * 2 * PTW], BF16,
                           kind="ExternalInput")
    # w1p: [128, 9 kij, 4 cb, 256 m]
    w1p = nc.dram_tensor("w1p", [128, 9, 4 * 256], BF16, kind="ExternalInput")
    w2p = nc.dram_tensor("w2p", [128, 18, 128], BF16, kind="ExternalInput")
    # conv2 boundary-class row sums: [(class4, cb2) blocks][c1w, c2]
    w2sp = nc.dram_tensor("w2sp", [128, 8 * 128], BF16, kind="ExternalInput")
    weffp = nc.dram_tensor("weffp", [128, 26], F32, kind="ExternalInput")
    bprep = nc.dram_tensor("bprep", [128, 7], F32, kind="ExternalInput")
    out = nc.dram_tensor("out", [BL, 1], F32, kind="ExternalOutput")

    # bprep columns: bn1_g (2), bn1_b (2), bn2_g, bn2_b, beff(row 0)
    BC_BN1G, BC_BN1B, BC_BN2G, BC_BN2B, BC_BEFF = 0, 2, 4, 5, 6

    ISQ1 = 1.0 / np.sqrt(N1)   # Square-accum scale so accum = sum(h^2)/N1
    ISQ2 = 1.0 / np.sqrt(N2)

    with tile.TileContext(nc) as tc:
        with tc.tile_pool(name="wp", bufs=1) as wp, \
             tc.tile_pool(name="xp", bufs=5) as xp, \
             tc.tile_pool(name="hp", bufs=1) as hp, \
             tc.tile_pool(name="sp", bufs=1) as sp, \
             tc.tile_pool(name="cps", bufs=8, space="PSUM") as cps, \
             tc.tile_pool(name="dram", bufs=1, space="DRAM") as dram:

            # ---------------- first loads ------------------------------
            # Issue order: x(k0,h0) then w1(k0) so the first matmul is gated
            # only on ~520KB of cold DMA.
            w1sb = wp.tile([128, 9 * 1024], BF16)
            xt = {}
            for half in range(2):
                for k in range(9):
                    xt[(k, half)] = xp.tile([128, 8 * PTW], BF16,
                                            name=f"xt{k}_{half}", tag="xt")
            nc.sync.dma_start(xt[(0, 0)][:, 0:4 * PTW], xprep.ap()[0, 0][:, 0:4 * PTW])
            w1r = w1p.ap().rearrange("p a b -> p (a b)")
            nc.sync.dma_start(w1sb[:, 0:1024], w1r[:, 0:1024])
            nc.sync.dma_start(xt[(0, 0)][:, 4 * PTW:], xprep.ap()[0, 0][:, 4 * PTW:])

            # ncfw warm-up: a tiny AllGather nobody consumes; hides the
            # TOPSP cold-start under conv1
            warm_in = dram.tile([1, 4], F32)
            warm_out = dram.tile([NCORES, 1, 4], F32, addr_space="Shared")
            dummy = sp.tile([1, 8], F32)
            nc.gpsimd.memset(dummy[:], 0.0)
            epst = sp.tile([128, 1], F32)
            nc.gpsimd.memset(epst[:], EPS)
            nc.scalar.dma_start(warm_in[:], dummy[:, 0:4])
            nc.gpsimd.collective_compute(
                "AllGather", mybir.AluOpType.bypass,
                replica_groups=[list(range(NCORES))],
                ins=[warm_in.opt()], outs=[warm_out.opt()])
            # ACT Square table preload while ACT is idle
            nc.scalar.activation(dummy[:, 4:5], dummy[:, 5:6],
                                 mybir.ActivationFunctionType.Square)

            # remaining streamed loads, interleaved with compute demand order
            for k in range(1, 9):
                nc.sync.dma_start(xt[(k, 0)][:], xprep.ap()[k, 0])
                nc.sync.dma_start(w1sb[:, k * 1024:(k + 1) * 1024],
                                  w1r[:, k * 1024:(k + 1) * 1024])
            for k in range(9):
                nc.sync.dma_start(xt[(k, 1)][:], xprep.ap()[k, 1])
            w2sb = wp.tile([128, 18 * 128], BF16)
            nc.sync.dma_start(w2sb[:], w2p.ap().rearrange("p a b -> p (a b)"))
            w2s_sb = wp.tile([128, 8 * 128], BF16)
            nc.sync.dma_start(w2s_sb[:], w2sp.ap())
            weff = wp.tile([128, 26], F32)
            nc.sync.dma_start(weff[:], weffp.ap())
            bsb = wp.tile([128, 7], F32)
            nc.sync.dma_start(bsb[:], bprep.ap())

            # ---------------- conv1 (k-major per half) ------------------
            scratch = sp.tile([128, PTW], F32)
            # h1 tiles hold RAW conv1 output in zero-padded 5*5*8 blocks
            # per (ki,kj); border slots stay zero (memset once)
            h1sb = [hp.tile([128, H1W], BF16, name=f"h1_{mt}")
                    for mt in range(2)]
            for mt in range(2):
                nc.gpsimd.memset(h1sb[mt][:], 0.0)
            # per-psum-tile stats: [128, 4pt * (2kind*2mt)]; kind0=sum, 1=sumsq
            stt = sp.tile([128, 16], F32)

            for half in range(2):
                ps = {}
                for ptin in range(2):
                    for mt in range(2):
                        ps[(ptin, mt)] = cps.tile([128, PTW], F32,
                                                  name=f"c1ps{half}{ptin}{mt}",
                                                  tag="c1ps")
                for k in range(9):
                    xk = xt[(k, half)][:].rearrange(
                        "p (c t w) -> p c t w", c=4, t=2)
                    for cb in range(4):
                        for mt in range(2):
                            lhsT = w1sb[:, (k * 4 + cb) * 256 + mt * 128:
                                        (k * 4 + cb) * 256 + (mt + 1) * 128]
                            for ptin in range(2):
                                nc.tensor.matmul(
                                    ps[(ptin, mt)][:], lhsT, xk[:, cb, ptin],
                                    start=(k == 0 and cb == 0),
                                    stop=(k == 8 and cb == 3))
                # stats first (they gate the AllGather trigger; the casts
                # below can drain during the collective): sum on DVE,
                # sum(h^2)/N1 via ACT Square-accumulate, straight off PSUM
                for ptin in range(2):
                    pt = half * 2 + ptin
                    for mt in range(2):
                        p = ps[(ptin, mt)]
                        nc.vector.reduce_sum(stt[:, pt * 4 + mt:pt * 4 + mt + 1],
                                             p[:], axis=mybir.AxisListType.X)
                        nc.scalar.activation(scratch[:], p[:],
                                             mybir.ActivationFunctionType.Square,
                                             scale=ISQ1,
                                             accum_out=stt[:, pt * 4 + 2 + mt:
                                                           pt * 4 + 3 + mt])
                for ptin in range(2):
                    pt = half * 2 + ptin
                    for mt in range(2):
                        p = ps[(ptin, mt)]
                        # im2col blocks for conv2, cast to bf16
                        pr = p[:].rearrange("p (n i j) -> p n i j", n=2, i=14,
                                            j=14)
                        for kidx, (ki, kj) in enumerate(KIJ9):
                            ilo, icnt = (1, 4) if ki == 0 else (0, 5)
                            jlo, jcnt = (1, 4) if kj == 0 else (0, 5)
                            srcv = pr[:, :, 3 * ilo + ki - 1:14:3,
                                      3 * jlo + kj - 1:14:3].transpose([0, 2, 3, 1])
                            off = kidx * BLK + (ilo * 5 + jlo) * 8
                            dstv = bass.AP(
                                h1sb[mt].tensor,
                                h1sb[mt].offset + off + 2 * pt,
                                [list(h1sb[mt].ap[0]), [40, icnt],
                                 [8, jcnt], [1, 2]])
                            nc.vector.tensor_copy(dstv, srcv)

            # ---------------- BN1 stats combine + AllGather -------------
            # combine 4 pt blocks: [128, 4pt, 4] -> [128, 4]
            nc.vector.tensor_tensor(stt[:, 0:8], stt[:, 0:8], stt[:, 8:16],
                                    op=mybir.AluOpType.add)
            nc.vector.tensor_tensor(stt[:, 0:4], stt[:, 0:4], stt[:, 4:8],
                                    op=mybir.AluOpType.add)
            st1 = stt[:, 0:4]    # [S_mt0, S_mt1, Q_mt0, Q_mt1]; Q pre-div N1
            bn1_in = dram.tile([128, 4], F32)
            bn1_out = dram.tile([NCORES, 128, 4], F32, addr_space="Shared")
            nc.scalar.dma_start(bn1_in[:], st1)
            nc.gpsimd.collective_compute(
                "AllGather", mybir.AluOpType.bypass,
                replica_groups=[list(range(NCORES))],
                ins=[bn1_in.opt()], outs=[bn1_out.opt()])
            # Sqrt table preload during the AllGather wait; the stt read
            # keeps it ordered after the stats Squares, scale=0 + eps bias
            # keeps the argument in sqrt's valid range
            nc.scalar.activation(dummy[:, 4:5], stt[0:1, 0:1],
                                 mybir.ActivationFunctionType.Sqrt,
                                 scale=0.0, bias=epst[0:1, 0:1])
            stg = sp.tile([128, NCORES * 4], F32)
            nc.scalar.dma_start(
                stg[:].rearrange("p (r t) -> p r t", r=NCORES),
                bass.AP(bn1_out.tensor, 0, [[4, 128], [128 * 4, NCORES], [1, 4]]))
            stgr = stg[:].rearrange("p (r t) -> p r t", r=NCORES)
            for halfn in (4, 2, 1):
                nc.vector.tensor_tensor(
                    stgr[:, 0:halfn], stgr[:, 0:halfn],
                    stgr[:, halfn:2 * halfn], op=mybir.AluOpType.add)

            # ---------------- BN coeffs helper --------------------------
            def bn_coeffs(pool, stats_sum, stats_sqn, count, g_ap, b_ap, name):
                """stats_sum raw sum; stats_sqn = sum(h^2)/count.
                Returns (scale, shift) [p,w]."""
                p, w = stats_sum.shape
                t = pool.tile([p, 4 * w], F32, name=f"bn_{name}")
                mean, var, sd, tn = (t[:, i * w:(i + 1) * w] for i in range(4))
                nc.vector.tensor_scalar(mean, stats_sum, 1.0 / count, None,
                                        op0=mybir.AluOpType.mult)
                nc.vector.tensor_tensor(var, mean, mean,
                                        op=mybir.AluOpType.mult)
                nc.vector.tensor_tensor(var, stats_sqn, var,
                                        op=mybir.AluOpType.subtract)
                nc.scalar.activation(sd, var,
                                     mybir.ActivationFunctionType.Sqrt,
                                     bias=epst[:, 0:1])
                co = pool.tile([p, 3 * w], F32, name=f"bnc_{name}")
                scale, shift, r = co[:, 0:w], co[:, w:2 * w], co[:, 2 * w:3 * w]
                nc.vector.reciprocal(r, sd)
                nc.vector.tensor_tensor(scale, g_ap, r, op=mybir.AluOpType.mult)
                nc.vector.tensor_tensor(tn, mean, scale,
                                        op=mybir.AluOpType.mult)
                nc.vector.tensor_tensor(shift, b_ap, tn,
                                        op=mybir.AluOpType.subtract)
                return scale, shift

            scale1, shift1 = bn_coeffs(
                sp, stg[:, 0:2], stg[:, 2:4], N1,
                bsb[:, BC_BN1G:BC_BN1G + 2], bsb[:, BC_BN1B:BC_BN1B + 2], "bn1")
            # BN1 is folded into conv2: the per-input-channel scale goes
            # into w2 (h1sb's zero padding must stay zero), the shift's
            # contribution is a per-boundary-class constant computed by
            # 8 tiny matmuls against the host-precomputed class row sums.
            for cb2 in range(2):
                nc.vector.tensor_scalar(
                    w2sb[:, cb2 * 1152:(cb2 + 1) * 1152],
                    w2sb[:, cb2 * 1152:(cb2 + 1) * 1152],
                    scale1[:, cb2:cb2 + 1], None, op0=mybir.AluOpType.mult)
            shift1b = sp.tile([128, 2], BF16)
            nc.vector.tensor_copy(shift1b[:], shift1)
            Tm = cps.tile([128, 4], F32, tag="c1ps")
            for cls in range(4):
                for cb2 in range(2):
                    nc.tensor.matmul(Tm[:, cls:cls + 1],
                                     w2s_sb[:, (cls * 2 + cb2) * 128:
                                            (cls * 2 + cb2 + 1) * 128],
                                     shift1b[:, cb2:cb2 + 1],
                                     start=(cb2 == 0), stop=(cb2 == 1),
                                     skip_group_check=True)
            TmS = sp.tile([128, 4], F32)
            nc.vector.tensor_copy(TmS[:], Tm[:])

            # ---------------- conv2 (one 18-matmul chain) ---------------
            c2p = cps.tile([128, P2 * BL], F32, name="c2p", tag="c1ps")
            for idx in range(18):
                cb2, kidx = idx // 9, idx % 9
                lhsT = w2sb[:, idx * 128:(idx + 1) * 128]
                nc.tensor.matmul(c2p[:], lhsT,
                                 h1sb[cb2][:, kidx * BLK:(kidx + 1) * BLK],
                                 start=(idx == 0), stop=(idx == 17))
            # psum -> sbuf bf16, adding the BN1-shift boundary-class consts
            c2sb = sp.tile([128, BL * P2], BF16)
            c2vp = c2p[:].rearrange("p (i j n) -> p i j n", i=5, j=5, n=BL)
            c2v = c2sb[:].rearrange("p (i j n) -> p i j n", i=5, j=5, n=BL)
            for cls, sl in ((3, (slice(0, 1), slice(0, 1))),
                            (2, (slice(0, 1), slice(1, 5))),
                            (1, (slice(1, 5), slice(0, 1))),
                            (0, (slice(1, 5), slice(1, 5)))):
                nc.vector.tensor_scalar(c2v[:, sl[0], sl[1]],
                                        c2vp[:, sl[0], sl[1]],
                                        TmS[:, cls:cls + 1], None,
                                        op0=mybir.AluOpType.add)

            # ---------------- BN2 stats + AllGather (before matvec) -----
            st2l = sp.tile([128, 2], F32)
            nc.vector.reduce_sum(st2l[:, 0:1], c2sb[:], axis=mybir.AxisListType.X)
            sc2 = sp.tile([128, BL * P2], F32)
            nc.scalar.activation(sc2[:], c2sb[:],
                                 mybir.ActivationFunctionType.Square,
                                 scale=ISQ2, accum_out=st2l[:, 1:2])
            bn2_in = dram.tile([128, 2], F32)
            bn2_out = dram.tile([NCORES, 128, 2], F32, addr_space="Shared")
            nc.scalar.dma_start(bn2_in[:], st2l[:])
            nc.gpsimd.collective_compute(
                "AllGather", mybir.AluOpType.bypass,
                replica_groups=[list(range(NCORES))],
                ins=[bn2_in.opt()], outs=[bn2_out.opt()])

            # during the AllGather: weff partial matvec + Sigmoid table +
            # the pieces of the finish that don't need stats
            mvt = sp.tile([128, P2 * BL], F32)
            wb = weff[:, 0:25, None].to_broadcast([128, 25, BL])
            nc.vector.tensor_tensor(
                mvt[:].rearrange("p (i n) -> p i n", i=P2),
                c2sb[:].rearrange("p (i n) -> p i n", i=P2), wb,
                op=mybir.AluOpType.mult)
            Av = sp.tile([128, BL], F32)
            nc.vector.reduce_sum(Av[:], mvt[:].rearrange("p (i n) -> p n i", i=P2),
                                 axis=mybir.AxisListType.X)
            Avb = sp.tile([128, BL], BF16)
            nc.vector.tensor_copy(Avb[:], Av[:])
            nc.scalar.activation(dummy[:, 4:5], st2l[0:1, 0:1],
                                 mybir.ActivationFunctionType.Sigmoid)
            ones = wp.tile([128, BL], BF16)
            nc.gpsimd.memset(ones[:], 1.0)

            stg2 = sp.tile([128, NCORES * 2], F32)
            nc.scalar.dma_start(
                stg2[:].rearrange("p (r t) -> p r t", r=NCORES),
                bass.AP(bn2_out.tensor, 0, [[2, 128], [128 * 2, NCORES], [1, 2]]))
            stg2r = stg2[:].rearrange("p (r t) -> p r t", r=NCORES)
            for halfn in (4, 2, 1):
                nc.vector.tensor_tensor(
                    stg2r[:, 0:halfn], stg2r[:, 0:halfn],
                    stg2r[:, halfn:2 * halfn], op=mybir.AluOpType.add)
            scale2, shift2 = bn_coeffs(
                sp, stg2[:, 0:1], stg2[:, 1:2], N2,
                bsb[:, BC_BN2G:BC_BN2G + 1], bsb[:, BC_BN2B:BC_BN2B + 1], "bn2")

            # ---------------- collapsed MLP finish ----------------------
            # z[n] = sum_c s2[c]*A[c,n] + sum_c shift2[c]*rowsum_weff[c]
            s2b = sp.tile([128, 1], BF16)
            nc.vector.tensor_copy(s2b[:], scale2)
            vsh = wp.tile([128, 1], BF16)
            nc.vector.tensor_tensor(vsh[:], shift2, weff[:, 25:26],
                                    op=mybir.AluOpType.mult)
            zps = cps.tile([1, BL], F32, tag="c1ps")
            nc.tensor.matmul(zps[:], s2b[:], Avb[:], start=True, stop=False)
            nc.tensor.matmul(zps[:], vsh[:], ones[:], start=False, stop=True)
            osb = sp.tile([1, BL], F32)
            nc.scalar.activation(osb[:], zps[:],
                                 mybir.ActivationFunctionType.Sigmoid,
                                 bias=bsb[0:1, BC_BEFF:BC_BEFF + 1])
            nc.sync.dma_start(bass.AP(out, 0, [[1, 1], [1, BL]]), osb[:])

    nc.compile()
    return nc


# ----------------------------------------------------------------------------
# host-side input prep
# ----------------------------------------------------------------------------

def _prep_inputs(inputs):
    import ml_dtypes
    f = np.float32
    bf = ml_dtypes.bfloat16
    x = np.asarray(inputs["x"], dtype=f)

    # conv1 patches: [n64, cb4, c128, i14, ki3, j14, kj3]
    xpad = np.zeros((B, 512, 42, 42), dtype=bf)
    xpad[:, :, 1:41, 1:41] = x.astype(bf)
    # -> [k9, cb4, c128, n64, pos196]
    xv = (xpad.reshape(B, 4, 128, 14, 3, 14, 3)
          .transpose(4, 6, 1, 2, 0, 3, 5)        # ki,kj,cb,c,n,i,j
          .reshape(9, 4, 128, B, P1))

    w1 = np.asarray(inputs["conv1_w"], dtype=f)          # [256, 512, 3, 3]
    # [128c, 9k, 4cb, 256m]
    w1p = np.ascontiguousarray(
        w1.reshape(256, 4, 128, 9).transpose(2, 3, 1, 0)).reshape(
            128, 9, 1024).astype(bf)
    w2 = np.asarray(inputs["conv2_w"], dtype=f)          # [128, 256, 3, 3]
    w2p = np.ascontiguousarray(
        w2.reshape(128, 2, 128, 9).transpose(2, 1, 3, 0)).reshape(
            128, 18, 128).astype(bf)
    # boundary-class row sums for the BN1-shift term:
    # class c = a*2+b, a=(i==0) -> ki>=1 only, b=(j==0) -> kj>=1 only
    w2r = w2.reshape(128, 2, 128, 3, 3)                  # c2, cb, c1w, ki, kj
    w2sp = np.zeros((128, 8 * 128), dtype=f)
    for cls in range(4):
        a, b = cls // 2, cls % 2
        kis = slice(1, 3) if a else slice(0, 3)
        kjs = slice(1, 3) if b else slice(0, 3)
        s = w2r[:, :, :, kis, kjs].sum(axis=(3, 4))      # c2, cb, c1w
        for cb in range(2):
            w2sp[:, (cls * 2 + cb) * 128:(cls * 2 + cb + 1) * 128] = s[:, cb].T
    w2sp = w2sp.astype(bf)

    # compose the 12 affine layers (no nonlinearities) into [3200] + scalar
    M = np.asarray(inputs["w14"], dtype=np.float64)      # [1, 2]
    beff = np.asarray(inputs["b14"], dtype=np.float64).copy()  # [1]
    for li in range(13, 2, -1):                          # w13 .. w3
        beff += M @ np.asarray(inputs[f"b{li}"], dtype=np.float64)
        M = M @ np.asarray(inputs[f"w{li}"], dtype=np.float64)
    weff = M.reshape(3200).astype(f)                     # order f = c*25 + ij
    w2d = weff.reshape(128, 25)
    weffp = np.zeros((128, 26), dtype=f)
    weffp[:, 0:25] = w2d
    weffp[:, 25] = w2d.sum(axis=1)
    beff_f = float(beff[0])

    bp = np.zeros((128, 7), dtype=f)
    bp[:, 0:2] = np.asarray(inputs["bn1_g"], dtype=f).reshape(2, 128).T
    bp[:, 2:4] = np.asarray(inputs["bn1_b"], dtype=f).reshape(2, 128).T
    bp[:, 4] = np.asarray(inputs["bn2_g"], dtype=f)
    bp[:, 5] = np.asarray(inputs["bn2_b"], dtype=f)
    bp[0, 6] = beff_f

    in_maps = []
    for r in range(NCORES):
        # [9k, 4cb, 128, 8n, 196] -> [9k, 2half, 128, 4cb, 2ptin, 2n, 196]
        xr = np.ascontiguousarray(
            xv[:, :, :, r * BL:(r + 1) * BL]
            .reshape(9, 4, 128, 2, 2, 2, P1)     # k, cb, c, half, ptin, n2, pos
            .transpose(0, 3, 2, 1, 4, 5, 6)
        ).reshape(9, 2, 128, 4 * 2 * PTW)
        in_maps.append({
            "xprep": xr, "w1p": w1p, "w2p": w2p, "w2sp": w2sp,
            "weffp": weffp, "bprep": bp,
        })
    return in_maps


def kernel(**inputs):
    if "nc" not in _CACHE:
        _CACHE["nc"] = _build()
    nc = _CACHE["nc"]
    in_maps = _prep_inputs(inputs)
    trace = bool(int(os.environ.get("KERNEL_TRACE", "0")))
    if trace:
        try:
            import ntff_shim
            ntff_shim.install()
        except ImportError:
            trace = False
    res = run_bass_kernel_spmd(nc, in_maps, core_ids=list(range(NCORES)),
                               trace=trace)
    _CACHE["last_result"] = res
    return np.concatenate([res.results[r]["out"] for r in range(NCORES)], axis=0)
